# revision 50
# baseline (speedup 1.0000x reference)
"""Trainium2 Bass kernel for spatial multi-head attention (gather-attention).

Computation (per agent b, H=8 heads, DK=32, K=32 neighbors, NB=16384):
    q = query @ Wq.T + bq ; k = query @ Wk.T (+bk) ; v = query @ Wv.T (+bv)
    s[b,h,k] = q[b,h,:] . k[nbr[b,k],h,:] / sqrt(DK)   (masked softmax over k)
    x[b,h,:] = sum_k p[b,h,k] v[nbr[b,k],h,:]
    out      = x @ Wo.T + bo

Key algebraic simplifications (exact):
  - bk drops out (softmax invariant to per-(b,h) constants)
  - bv folds into the output bias: out += (bo + Wo @ bv)  (softmax sums to 1)
  - V is NEVER materialized: by linearity of v = Wv x,
        x_out_h[b] = sum_k p[b,h,k] Wv_h x_{j_k} = Wv_h u_h[b],
        u_h[b] = sum_k p[b,h,k] x_{j_k}          (raw input rows!)
        out[b] = sum_h G_h u_h[b] + boeff,  G_h = Wo_h @ Wv_h  (host precomputed)
    so the gather fetches raw x rows instead of projected V rows and the
    whole V projection + output projection collapse into 16 G-matmuls.

Device strategy (8 cores, data-parallel over agents, 2048 agents/core):
  - Each core computes the FULL fp16 K table, pairs it with the raw fp16 x
    rows into ONE interleaved DRAM table kxtab[16384, 512] (1KB rows:
    [K_j (256 f16) || x_j (256 f16)]).
  - Per 128-agent chunk ONE non-transpose dma_gather (4096 idx, 1KB rows)
    fetches both K and x. Non-transpose gathers have no SBUF-crossbar
    serialization hazard, so chunks rotate over all 4 SWDGE queues. Merging
    K and x into one row halves descgen work (descgen cost is ~8ns per
    INDEX regardless of row bytes; the old design's serial K transpose
    chain alone was 65536 idx * 8ns = 524us).
  - Gathered row i lands at partition i%128, slot i//128; host index
    permutation i=(m*128+a*32+k) -> agent ch*128+m*4+a, neighbor k puts
    the data at [p=(a,k), m, :]: 4 agents * 32 neighbors per partition dim.
  - Scores: K halves are transposed on-chip by PE (nc.tensor.transpose,
    f16 PSUM out), multiplied by q (DVE, q indexed by d on partitions ->
    trivial broadcast), and block-reduced per head by PE with a constant
    block-ones rhs: sc[p=(a,k), m, h].
  - Softmax: additive mask + exp (Act), denominator via ONE PE matmul with
    a block-replication matrix (z broadcast over k for free), reciprocal
    straight to f16 (DVE), normalize.
  - u: per (m, d-half) PE matmul contracts the (a,k) partition dim of the
    gathered x rows against a block-diagonalized P (Pdiag), output lands
    [d', (a,h)] in PSUM; Act unloads to f16.
  - out: 16 accumulating PE matmuls against host-precomputed G_h^T + bias.
  - Emission is software-pipelined (skewed): rest(t-2), tp(t-1), gather(t).
    Per-engine stream ORDER is critical: each engine finishes chunk c's
    consumer work (softmax/u/G) before chunk c+1's producer work
    (transposes/prod), else the in-order streams serialize into a ~30us/
    chunk zigzag.
  - kx bufs=4 (the SBUF limit): the Pool engine's in-order wait queue
    HEAD-BLOCKS gather descgen on the kx tile's WAR edge (gather(ch) waits
    for chunk ch-4's u-matmuls), so the steady cadence is
    (descgen 32us + rx 12us + compute chain ~45us)/4 = ~20us/chunk.
    Deeper buffering would help but does not fit; splitting K/x into two
    gathers doubles per-idx descgen and is a net loss (measured).

Hard-won HW facts (verified on this device):
  - prepare_only + trigger_dma is BROKEN end-to-end: with count=None the
    gathered data is garbage; the documented explicit-sem pattern
    (then_inc/wait_ge/trigger count=1) wedges the device (NRT timeout),
    and wait_ge(dma_sem, 32) deadlocks even the build-time sim. Avoid.
  - single_packet=True on these gathers crashes the device
    (NRT_EXEC_UNIT_UNRECOVERABLE).
  - fp8e4 K table fails accuracy: rel err 3.2e-2 > 2e-2 gate (simulated).
  - DVE tensor ops from f16 PSUM run ~1.3ns/elem/partition (no 2x f16
    speedup; that needs SBUF sources with inner-broadcast patterns).
"""

import sys

if "/opt/trn_rl_repo" not in sys.path:
    sys.path.insert(0, "/opt/trn_rl_repo")

import numpy as np
from contextlib import ExitStack

H, DKD, DM = 8, 32, 256
KN = 32  # neighbors per agent
NB_FULL = 16384
NCORES = 8
SCALE = 1.0 / np.sqrt(DKD)
MASK_NEG = -1.0e9

_PROGRAM_CACHE = {}


def _build_program(NB, NBS, repeats=1):
    """Build the per-core Bass/Tile program. Identical across cores; all
    core-varying information comes in through input tensors."""
    import concourse.bacc as bacc
    import concourse.tile as tile
    import concourse.mybir as mybir
    from concourse.tile_rust import add_dep_helper
    from concourse.library_config import mlp as mlp_lib

    f16 = mybir.dt.float16
    f32 = mybir.dt.float32
    i16 = mybir.dt.int16
    Act = mybir.ActivationFunctionType

    CH = NBS // 128        # chunks of 128 agents
    NBT = NB // 128        # table row-tiles
    WG = min(8, NBT)       # table tiles batched per DRAM write
    IDXC = NBS * KN // 16  # idx tensor columns ([16, IDXC] wrapped)
    ROW = 2 * DM           # kxtab row: K(256 f16) || x(256 f16)

    nc = bacc.Bacc(
        "TRN2",
        target_bir_lowering=False,
        debug=False,
        num_swdge_queues=4,
        dynamic_dma_scratch_size=12288,
    )

    # ---- external inputs (host-prepped layouts) ----
    qT = nc.dram_tensor("qT", [DM, NB], f16, kind="ExternalInput").ap()
    qTs = nc.dram_tensor("qTs", [DM, NBS], f16, kind="ExternalInput").ap()
    WqT = nc.dram_tensor("WqT", [DM, DM], f16, kind="ExternalInput").ap()
    WkT = nc.dram_tensor("WkT", [DM, DM], f16, kind="ExternalInput").ap()
    GTd = nc.dram_tensor("GTd", [DM, H * DM], f16, kind="ExternalInput").ap()
    bqv = nc.dram_tensor("bqv", [DM, 1], f32, kind="ExternalInput").ap()
    boeff = nc.dram_tensor("boeff", [1, DM], f16, kind="ExternalInput").ap()
    ones4 = nc.dram_tensor("ones4", [128, 4], f16, kind="ExternalInput").ap()
    ones1 = nc.dram_tensor("ones1", [1, 128], f16, kind="ExternalInput").ap()
    blk128 = nc.dram_tensor("blk128", [128, 128], f16, kind="ExternalInput").ap()
    blk4 = nc.dram_tensor("blk4", [128, 4], f16, kind="ExternalInput").ap()
    ident = nc.dram_tensor("ident", [128, 128], f16, kind="ExternalInput").ap()
    idxA = nc.dram_tensor("idxA", [128, IDXC], i16, kind="ExternalInput").ap()
    maskA = nc.dram_tensor("maskA", [128, CH * 32], f32, kind="ExternalInput").ap()
    outp = nc.dram_tensor("out", [NBS, DM], f32, kind="ExternalOutput").ap()

    # ---- internal DRAM K||x table (fp16 1KB rows, gather source) ----
    kxtab = nc.dram_tensor("kxtab", [NB, ROW], f16).ap()

    with tile.TileContext(nc) as tc:
      for _rep in range(repeats):
        with ExitStack() as ctx:
            libload = nc.gpsimd.load_library(mlp_lib)
            consts = ctx.enter_context(tc.tile_pool(name="consts", bufs=1))

            def load_const(name, ap, shape, dtype, rearr=None):
                t = consts.tile(shape, dtype, tag=name)
                src = ap if rearr is None else ap.rearrange(rearr, p=128)
                nc.sync.dma_start(t[:], src)
                return t

            wq_sb = load_const("wq", WqT, [128, 2, DM], f16, "(c p) d -> p c d")
            wk_sb = load_const("wk", WkT, [128, 2, DM], f16, "(c p) d -> p c d")
            gt_sb = load_const("gt", GTd, [128, 2, H * DM], f16, "(c p) d -> p c d")
            bq_sb = load_const("bq", bqv, [128, 2], f32, "(c p) o -> p (c o)")
            bo_sb = load_const("bo", boeff, [1, DM], f16)
            on4_sb = load_const("on4", ones4, [128, 4], f16)
            on1_sb = load_const("on1", ones1, [1, 128], f16)
            bk128_sb = load_const("bk128", blk128, [128, 128], f16)
            bk4_sb = load_const("bk4", blk4, [128, 4], f16)
            id_sb = load_const("ident", ident, [128, 128], f16)
            idx_sb = load_const("idx", idxA, [128, IDXC], i16)
            msk_sb = load_const("msk", maskA, [128, CH * 32], f32)

            qtn = ctx.enter_context(tc.tile_pool(name="qtn", bufs=1))
            qTn_sb = qtn.tile([128, 2, NBS], f16, tag="qTn")

            # ---------------- Phase A: K table + x transpose ----------------
            with ExitStack() as actx:
                qpool = actx.enter_context(tc.tile_pool(name="qtp", bufs=1))
                qt_sb = qpool.tile([128, 2, NB], f16, tag="qt")
                # split load: table group g only needs qt cols up to its rows,
                # so the first matmuls start after 1/4 of the load
                qr = qT.rearrange("(c p) b -> p c b", p=128)
                for lq in range(4):
                    sl = slice(lq * NB // 4, (lq + 1) * NB // 4)
                    nc.sync.dma_start(qt_sb[:, :, sl], qr[:, :, sl])
                qs_sb = qpool.tile([128, 2, NBS], f16, tag="qs")
                nc.sync.dma_start(qs_sb[:], qTs.rearrange("(c p) b -> p c b", p=128))

                aps = actx.enter_context(
                    tc.tile_pool(name="aps", bufs=2, space="PSUM")
                )
                astg = actx.enter_context(tc.tile_pool(name="astg", bufs=4))

                # kxtab[b, 0:256] = K rows (qt.T @ Wk.T); [b, 256:512] = x rows
                # (PE transpose of qt). Batched group writes.
                kxwr = []
                for g in range(NBT // WG):
                    kstg = astg.tile([128, WG, ROW], f16, tag="kstg")
                    for j in range(0, WG, 2):
                        kp = aps.tile([128, 2, DM], f32, tag="kp")
                        for u2 in range(2):
                            bt = g * WG + j + u2
                            for ih in range(2):
                                nc.tensor.matmul(
                                    kp[:, u2, :],
                                    lhsT=qt_sb[:, ih, bt * 128 : bt * 128 + 128],
                                    rhs=wk_sb[:, ih, :],
                                    start=(ih == 0),
                                    stop=(ih == 1),
                                )
                        nc.vector.tensor_copy(kstg[:, j : j + 2, 0:DM], kp[:])
                    for j in range(WG):
                        bt = g * WG + j
                        xp2 = aps.tile([128, 2, 128], f16, tag="xp2")
                        for c in range(2):
                            nc.tensor.transpose(
                                xp2[:, c, :],
                                qt_sb[:, c, bt * 128 : bt * 128 + 128],
                                id_sb[:],
                            )
                        nc.scalar.copy(kstg[:, j, DM:ROW], xp2[:])
                    rows = 128 * WG
                    kxwr.append(
                        nc.sync.dma_start(
                            kxtab[g * rows : (g + 1) * rows, :].rearrange(
                                "(j p) d -> p j d", p=128
                            ),
                            kstg[:],
                        )
                    )

                # q_T projection: qTn[d, b] = Wq @ qTs + bq  (fp16, d on
                # parts). After the table groups: it is only consumed by
                # phase B's prod muls, so it must not delay the table.
                for dh in range(2):
                    for bc in range(NBS // 512):
                        qp = aps.tile([128, 512], f32, tag="qproj")
                        for ih in range(2):
                            nc.tensor.matmul(
                                qp[:],
                                lhsT=wq_sb[:, ih, dh * 128 : dh * 128 + 128],
                                rhs=qs_sb[:, ih, bc * 512 : bc * 512 + 512],
                                start=(ih == 0),
                                stop=(ih == 1),
                            )
                        nc.scalar.activation(
                            qTn_sb[:, dh, bc * 512 : bc * 512 + 512],
                            qp[:],
                            Act.Identity,
                            bias=bq_sb[:, dh : dh + 1],
                            scale=1.0,
                        )

            # ---------------- Phase B: gather + attention chunks ----------------
            # kx bufs=4: the in-order Pool wait-queue head-blocks descgen on
            # the gather tiles' WAR edges; 4-deep keeps them stale so the 4
            # queues' descgens overlap.
            kxp = ctx.enter_context(tc.tile_pool(name="kxp", bufs=4))
            prp = ctx.enter_context(tc.tile_pool(name="prp", bufs=3))
            ktsp = ctx.enter_context(tc.tile_pool(name="ktsp", bufs=2))
            pdp = ctx.enter_context(tc.tile_pool(name="pdp", bufs=2))
            utp = ctx.enter_context(tc.tile_pool(name="utp", bufs=2))
            sfx = ctx.enter_context(tc.tile_pool(name="sfx", bufs=2))
            oup = ctx.enter_context(tc.tile_pool(name="oup", bufs=1))
            ktrp = ctx.enter_context(tc.tile_pool(name="ktrp", bufs=4, space="PSUM"))
            sczp = ctx.enter_context(tc.tile_pool(name="sczp", bufs=2, space="PSUM"))
            upsp = ctx.enter_context(tc.tile_pool(name="upsp", bufs=1, space="PSUM"))
            gop = ctx.enter_context(tc.tile_pool(name="gop", bufs=1, space="PSUM"))

            kx_t = [None] * CH
            ktr_t = [None] * CH
            pr_t = [None] * CH
            scz_t = [None] * CH

            def emit_gather(ch):
                # Two half-gathers (m 0-15 / 16-31) per chunk on different
                # queues: slice-level tile writes mean slice-level WAR
                # release, the compute chain starts on half 0 while half 1
                # is still landing, and descgens of the halves overlap.
                kx = kxp.tile([128, KN, ROW], f16, tag="kx")
                kx_t[ch] = kx
                for hf in range(2):
                    gi = nc.gpsimd.dma_gather(
                        kx[:, hf * 16 : hf * 16 + 16, :],
                        kxtab,
                        idx_sb[:, ch * 256 + hf * 128 : ch * 256 + hf * 128 + 128],
                        num_idxs=2048,
                        num_idxs_reg=2048,
                        elem_size=ROW,
                        transpose=False,
                        single_packet=False,
                        queue_num=(2 * ch + hf) % 4,
                    )
                    add_dep_helper(gi.ins, libload.ins, sync=True)
                    for w in kxwr:
                        add_dep_helper(gi.ins, w.ins, sync=True)

            def emit_tp(ch):
                # PE transpose of the K halves (f16 PSUM out) + DVE q-multiply.
                # ktr group layout: [p=d', dh, mi, (a,k)] for m = 4*g + mi.
                kx = kx_t[ch]
                ktr_t[ch] = []
                pr_t[ch] = []
                for g in range(8):
                    ktr = ktrp.tile([128, 2, 4, 128], f16, tag="ktr")
                    for dh in range(2):
                        for mi in range(4):
                            m_ = g * 4 + mi
                            nc.tensor.transpose(
                                ktr[:, dh, mi, :],
                                kx[:, m_, dh * 128 : dh * 128 + 128],
                                id_sb[:],
                            )
                    pr = prp.tile([128, 2, 4, 128], f16, tag="pr")
                    qsl = (
                        qTn_sb[:, :, ch * 128 + g * 16 : ch * 128 + g * 16 + 16]
                        .rearrange("p c (mi a u) -> p c mi a u", a=4, u=1)
                        .broadcast_to([128, 2, 4, 4, KN])
                    )
                    if g % 2:
                        # Act unloads PSUM so the DVE mul runs at the 2x f16
                        # SBUF rate (f16-PSUM reads are ~2.5x slower);
                        # alternating groups balances DVE vs Act.
                        kts = ktsp.tile([128, 2, 4, 128], f16, tag="kts")
                        nc.scalar.copy(kts[:], ktr[:])
                        ksrc = kts
                    else:
                        ksrc = ktr
                    nc.vector.tensor_mul(
                        pr[:].rearrange("p c mi (a k) -> p c mi a k", k=KN),
                        ksrc[:].rearrange("p c mi (a k) -> p c mi a k", k=KN),
                        qsl,
                    )
                    ktr_t[ch].append(ktr)
                    pr_t[ch].append(pr)

            def emit_rest(ch):
                # Per m-HALF sub-chains: softmax is over k (within a
                # partition), so m 0-15 completes scores->softmax->u without
                # waiting for m 16-31. This halves the chain latency that
                # gates the gather-tile WAR release.
                uT = utp.tile([128, 2, H, KN, 4], f16, tag="uT")
                kx = kx_t[ch]
                for half in range(2):
                    # scores: PE block-reduce over d (32-part blocks = heads)
                    scz = sczp.tile([128, 2, 16 * H], f32, tag="scz")
                    scv = scz[:, 0, :].rearrange("p (m h) -> p m h", h=H)
                    for g in range(4):
                        pr = pr_t[ch][half * 4 + g]
                        for dh in range(2):
                            for mi in range(4):
                                nc.tensor.matmul(
                                    scv[:, g * 4 + mi, dh * 4 : dh * 4 + 4],
                                    lhsT=pr[:, dh, mi, :],
                                    rhs=on4_sb[:, 0:4],
                                    start=True,
                                    stop=True,
                                )

                    # masked softmax (un-normalized exp, PE sum-bcast, recip)
                    sm = sfx.tile([128, 16, H], f32, tag="sf32")
                    mv = (
                        msk_sb[:, ch * 32 + half * 16 : ch * 32 + half * 16 + 16]
                        .rearrange("p (s u) -> p s u", u=1)
                        .broadcast_to([128, 16, H])
                    )
                    nc.vector.tensor_add(sm[:], scv, mv)
                    ex = sfx.tile([128, 16, H], f16, tag="ex")
                    nc.scalar.activation(ex[:], sm[:], Act.Exp, scale=float(SCALE))

                    nc.tensor.matmul(
                        scz[:, 1, :],
                        lhsT=bk128_sb[:],
                        rhs=ex[:].rearrange("p m h -> p (m h)"),
                        start=True,
                        stop=True,
                    )
                    rz16 = sfx.tile([128, 16 * H], f16, tag="rz16")
                    with nc.allow_low_precision(reason="softmax 1/z fits f16"):
                        nc.vector.reciprocal(rz16[:], scz[:, 1, :])
                    pn = sfx.tile([128, 16, H], f16, tag="pn")
                    nc.vector.tensor_mul(
                        pn[:], ex[:], rz16[:].rearrange("p (m h) -> p m h", h=H)
                    )

                    # Pdiag[p=(a,k), mm, a', h] = pn[p, mm, h] * (a == a')
                    pd = pdp.tile([128, 16, 4, H], f16, tag="pd")
                    nc.vector.tensor_mul(
                        pd[:],
                        pn[:].rearrange("p m (u h) -> p m u h", u=1).broadcast_to(
                            [128, 16, 4, H]
                        ),
                        bk4_sb[:]
                        .rearrange("p (u a v) -> p u a v", u=1, v=1)
                        .broadcast_to([128, 16, 4, H]),
                    )

                    # u[d', (a',h)] per (m, dh): contract (a,k) partitions
                    for dh in range(2):
                        ups = upsp.tile([128, 16, 32], f32, tag="ups")
                        for mm in range(16):
                            m_ = half * 16 + mm
                            nc.tensor.matmul(
                                ups[:, mm, :],
                                lhsT=kx[:, m_, DM + dh * 128 : DM + dh * 128 + 128],
                                rhs=pd[:, mm, :, :],
                                start=True,
                                stop=True,
                            )
                        nc.scalar.copy(
                            uT[:, dh, :, half * 16 : half * 16 + 16, :],
                            ups[:].rearrange("p s (a h) -> p h s a", h=H),
                        )

                # out = sum_h G_h u_h + boeff  (16 accumulating matmuls)
                go = gop.tile([128, DM], f32, tag="go")
                first = True
                for dh in range(2):
                    for h in range(H):
                        nc.tensor.matmul(
                            go[:],
                            lhsT=uT[:, dh, h, :, :],
                            rhs=gt_sb[:, dh, h * DM : h * DM + DM],
                            start=first,
                            stop=False,
                            skip_group_check=True,
                        )
                        first = False
                nc.tensor.matmul(
                    go[:],
                    lhsT=on1_sb[:],
                    rhs=bo_sb[:],
                    start=False,
                    stop=True,
                    skip_group_check=True,
                )
                ou = oup.tile([128, DM], f32, tag="ou")
                nc.scalar.copy(ou[:], go[:])
                nc.sync.dma_start(outp[ch * 128 : ch * 128 + 128, :], ou[:])

            # Skewed software pipeline. Per-engine stream order is what
            # matters: each engine must finish chunk c's CONSUMER work
            # (softmax/u/G on DVE+PE) before chunk c+1's PRODUCER work
            # (transposes/prod), else the streams serialize into a zigzag.
            for t in range(CH + 2):
                if t >= 2:
                    emit_rest(t - 2)
                if 1 <= t <= CH:
                    emit_tp(t - 1)
                if t < CH:
                    emit_gather(t)

    nc.compile()
    return nc


def _host_prep(query_, spatial_neighbors, mask, Wq, bq, Wk, bk, Wv, bv, Wo, bo,
               NB, NBS, ncores):
    """Pure-layout host prep: transposes, fp16 casts, index/mask relayout."""
    CH = NBS // 128
    f16 = np.float16

    q32 = np.asarray(query_, np.float32)
    qT16 = np.ascontiguousarray(q32.T).astype(f16)
    WqT16 = np.ascontiguousarray(np.asarray(Wq, np.float32).T).astype(f16)
    WkT16 = np.ascontiguousarray(np.asarray(Wk, np.float32).T).astype(f16)
    bq32 = np.asarray(bq, np.float32).reshape(DM, 1)
    boe = (np.asarray(bo, np.float64)
           + np.asarray(Wo, np.float64) @ np.asarray(bv, np.float64))
    boe16 = boe.astype(np.float32).astype(f16).reshape(1, DM)

    # GT[d_in, h*256 + o] = (Wo_h @ Wv_h)^T = G_h^T  (exact f64 product)
    Wo64 = np.asarray(Wo, np.float64)
    Wv64 = np.asarray(Wv, np.float64)
    GT = np.empty((DM, H * DM), np.float64)
    for h in range(H):
        Gh = Wo64[:, h * DKD : (h + 1) * DKD] @ Wv64[h * DKD : (h + 1) * DKD, :]
        GT[:, h * DM : (h + 1) * DM] = Gh.T
    GT16 = GT.astype(np.float32).astype(f16)

    blkcol = np.arange(128)[:, None] // 32 == np.arange(4)[None, :]
    ones4 = blkcol.astype(f16)                      # (p//32 == j)
    ones1 = np.ones((1, 128), f16)
    blk128 = (np.arange(128)[:, None] // 32
              == np.arange(128)[None, :] // 32).astype(f16)
    blk4 = blkcol.astype(f16)
    ident = np.eye(128, dtype=f16)

    nbr = np.asarray(spatial_neighbors, np.int64)
    msk = np.asarray(mask, np.int32).reshape(NB, KN)

    def wrap16(flat):
        # flat index i at [i%16, i//16], replicated 8x for the 8 Q7 cores
        return np.tile(flat.reshape(-1, 16).T, (8, 1)).astype(np.int16)

    # gather permutation: i_local = m*128 + a*32 + k -> agent m*4+a, nbr k
    i_loc = np.arange(NBS * KN)
    chv = i_loc // 4096
    r = i_loc % 4096
    m_, a_, k_ = r // 128, (r % 128) // 32, r % 32
    bV = chv * 128 + m_ * 4 + a_

    # additive mask layout [ (a,k) partitions, (ch, s) ]: agent ch*128+s*4+a
    pa, pk = np.arange(128) // 32, np.arange(128) % 32
    chs = np.arange(CH * 32) // 32
    ss = np.arange(CH * 32) % 32

    per_core = []
    for c in range(ncores):
        base = c * NBS
        sl = slice(base, base + NBS)
        qTs16 = np.ascontiguousarray(q32[sl].T).astype(f16)

        nbr_c = nbr[sl]
        iA = wrap16(nbr_c[bV, k_])      # permuted for (a,k)-partition layout

        bM = chs[None, :] * 128 + ss[None, :] * 4 + pa[:, None]  # [128, CH*32]
        mA = np.where(msk[sl][bM, pk[:, None]] != 0, 0.0, MASK_NEG).astype(np.float32)

        per_core.append(
            dict(
                qT=qT16, qTs=qTs16, WqT=WqT16, WkT=WkT16, GTd=GT16,
                bqv=bq32, boeff=boe16, ones4=ones4, ones1=ones1,
                blk128=blk128, blk4=blk4, ident=ident,
                idxA=iA, maskA=mA,
            )
        )
    return per_core


def kernel(**inputs):
    NB, NBS = NB_FULL, NB_FULL // NCORES
    key = (NB, NBS)
    if key not in _PROGRAM_CACHE:
        _PROGRAM_CACHE[key] = _build_program(NB, NBS)
    nc = _PROGRAM_CACHE[key]

    in_maps = _host_prep(NB=NB, NBS=NBS, ncores=NCORES, **inputs)

    from concourse.bass_utils import run_bass_kernel_spmd

    res = run_bass_kernel_spmd(nc, in_maps, list(range(NCORES)))
    out = np.concatenate([res.results[c]["out"] for c in range(NCORES)], axis=0)
    return out.reshape(NB, 1, DM).astype(np.float32)


# revision 54
# speedup vs baseline: 1.0502x; 1.0502x over previous
"""Trainium2 Bass kernel for spatial multi-head attention (gather-attention).

Computation (per agent b, H=8 heads, DK=32, K=32 neighbors, NB=16384):
    q = query @ Wq.T + bq ; k = query @ Wk.T (+bk) ; v = query @ Wv.T (+bv)
    s[b,h,k] = q[b,h,:] . k[nbr[b,k],h,:] / sqrt(DK)   (masked softmax over k)
    x[b,h,:] = sum_k p[b,h,k] v[nbr[b,k],h,:]
    out      = x @ Wo.T + bo

Key algebraic simplifications (exact):
  - bk drops out (softmax invariant to per-(b,h) constants)
  - bv folds into the output bias: out += (bo + Wo @ bv)  (softmax sums to 1)
  - V is NEVER materialized: by linearity of v = Wv x,
        x_out_h[b] = sum_k p[b,h,k] Wv_h x_{j_k} = Wv_h u_h[b],
        u_h[b] = sum_k p[b,h,k] x_{j_k}          (raw input rows!)
        out[b] = sum_h G_h u_h[b] + boeff,  G_h = Wo_h @ Wv_h  (host precomputed)
    so the gather fetches raw x rows instead of projected V rows and the
    whole V projection + output projection collapse into 16 G-matmuls.

Device strategy (8 cores, data-parallel over agents, 2048 agents/core):
  - Each core computes the FULL fp16 K table, pairs it with the raw fp16 x
    rows into ONE interleaved DRAM table kxtab[16384, 512] (1KB rows:
    [K_j (256 f16) || x_j (256 f16)]).
  - Per 128-agent chunk ONE non-transpose dma_gather (4096 idx, 1KB rows)
    fetches both K and x. Non-transpose gathers have no SBUF-crossbar
    serialization hazard, so chunks rotate over all 4 SWDGE queues. Merging
    K and x into one row halves descgen work (descgen cost is ~8ns per
    INDEX regardless of row bytes; the old design's serial K transpose
    chain alone was 65536 idx * 8ns = 524us).
  - Gathered row i lands at partition i%128, slot i//128; host index
    permutation i=(m*128+a*32+k) -> agent ch*128+m*4+a, neighbor k puts
    the data at [p=(a,k), m, :]: 4 agents * 32 neighbors per partition dim.
  - Scores: K halves are transposed on-chip by PE (nc.tensor.transpose,
    f16 PSUM out), multiplied by q (DVE, q indexed by d on partitions ->
    trivial broadcast), and block-reduced per head by PE with a constant
    block-ones rhs: sc[p=(a,k), m, h].
  - Softmax: additive mask + exp (Act), denominator via ONE PE matmul with
    a block-replication matrix (z broadcast over k for free), reciprocal
    straight to f16 (DVE), normalize.
  - u: per (m, d-half) PE matmul contracts the (a,k) partition dim of the
    gathered x rows against a block-diagonalized P (Pdiag), output lands
    [d', (a,h)] in PSUM; Act unloads to f16.
  - out: 16 accumulating PE matmuls against host-precomputed G_h^T + bias.
  - Emission is software-pipelined (skewed): rest(t-2), tp(t-1), gather(t).
    Per-engine stream ORDER is critical: each engine finishes chunk c's
    consumer work (softmax/u/G) before chunk c+1's producer work
    (transposes/prod), else the in-order streams serialize into a ~30us/
    chunk zigzag.
  - kx bufs=4 (the SBUF limit): the Pool engine's in-order wait queue
    HEAD-BLOCKS gather descgen on the kx tile's WAR edge (gather(ch) waits
    for chunk ch-4's u-matmuls), so the steady cadence is
    (descgen 32us + rx 12us + compute chain ~45us)/4 = ~20us/chunk.
    Deeper buffering would help but does not fit; splitting K/x into two
    gathers doubles per-idx descgen and is a net loss (measured).

Hard-won HW facts (verified on this device):
  - prepare_only + trigger_dma is BROKEN end-to-end: with count=None the
    gathered data is garbage; the documented explicit-sem pattern
    (then_inc/wait_ge/trigger count=1) wedges the device (NRT timeout),
    and wait_ge(dma_sem, 32) deadlocks even the build-time sim. Avoid.
  - single_packet=True on these gathers crashes the device
    (NRT_EXEC_UNIT_UNRECOVERABLE).
  - fp8e4 K table fails accuracy: rel err 3.2e-2 > 2e-2 gate (simulated).
  - DVE tensor ops from f16 PSUM run ~1.3ns/elem/partition (no 2x f16
    speedup; that needs SBUF sources with inner-broadcast patterns).
"""

import sys

if "/opt/trn_rl_repo" not in sys.path:
    sys.path.insert(0, "/opt/trn_rl_repo")

import numpy as np
from contextlib import ExitStack

H, DKD, DM = 8, 32, 256
KN = 32  # neighbors per agent
NB_FULL = 16384
NCORES = 8
SCALE = 1.0 / np.sqrt(DKD)
MASK_NEG = -1.0e9

_PROGRAM_CACHE = {}


def _build_program(NB, NBS, repeats=1):
    """Build the per-core Bass/Tile program. Identical across cores; all
    core-varying information comes in through input tensors."""
    import concourse.bacc as bacc
    import concourse.tile as tile
    import concourse.mybir as mybir
    from concourse.tile_rust import add_dep_helper
    from concourse.library_config import mlp as mlp_lib

    f16 = mybir.dt.float16
    f32 = mybir.dt.float32
    i16 = mybir.dt.int16
    Act = mybir.ActivationFunctionType

    CH = NBS // 128        # chunks of 128 agents
    NBT = NB // 128        # table row-tiles
    WG = min(8, NBT)       # table tiles batched per DRAM write
    IDXC = NBS * KN // 16  # idx tensor columns ([16, IDXC] wrapped)
    ROW = 2 * DM           # kxtab row: K(256 f16) || x(256 f16)

    nc = bacc.Bacc(
        "TRN2",
        target_bir_lowering=False,
        debug=False,
        num_swdge_queues=4,
        dynamic_dma_scratch_size=12288,
    )

    # ---- external inputs (host-prepped layouts) ----
    qT = nc.dram_tensor("qT", [DM, NB], f16, kind="ExternalInput").ap()
    qTs = nc.dram_tensor("qTs", [DM, NBS], f16, kind="ExternalInput").ap()
    WqT = nc.dram_tensor("WqT", [DM, DM], f16, kind="ExternalInput").ap()
    WkT = nc.dram_tensor("WkT", [DM, DM], f16, kind="ExternalInput").ap()
    GTd = nc.dram_tensor("GTd", [DM, H * DM], f16, kind="ExternalInput").ap()
    bqv = nc.dram_tensor("bqv", [DM, 1], f32, kind="ExternalInput").ap()
    boeff = nc.dram_tensor("boeff", [1, DM], f16, kind="ExternalInput").ap()
    ones4 = nc.dram_tensor("ones4", [128, 4], f16, kind="ExternalInput").ap()
    ones1 = nc.dram_tensor("ones1", [1, 128], f16, kind="ExternalInput").ap()
    blk128 = nc.dram_tensor("blk128", [128, 128], f16, kind="ExternalInput").ap()
    blk4 = nc.dram_tensor("blk4", [128, 4], f16, kind="ExternalInput").ap()
    ident = nc.dram_tensor("ident", [128, 128], f16, kind="ExternalInput").ap()
    idxA = nc.dram_tensor("idxA", [128, IDXC], i16, kind="ExternalInput").ap()
    maskA = nc.dram_tensor("maskA", [128, CH * 32], f32, kind="ExternalInput").ap()
    outp = nc.dram_tensor("out", [NBS, DM], f32, kind="ExternalOutput").ap()

    # ---- K||x table (fp16 1KB rows, gather source). The x half is staged
    # by the HOST (pure f16 cast of the input rows, no model flops); the
    # device computes and writes only the K half, halving phase A. The
    # device-side K writes are idempotent, so input-buffer reuse across
    # runs is safe. ----
    kxtab = nc.dram_tensor("kxtab", [NB, ROW], f16, kind="ExternalInput").ap()

    with tile.TileContext(nc) as tc:
      for _rep in range(repeats):
        with ExitStack() as ctx:
            libload = nc.gpsimd.load_library(mlp_lib)
            consts = ctx.enter_context(tc.tile_pool(name="consts", bufs=1))

            def load_const(name, ap, shape, dtype, rearr=None):
                t = consts.tile(shape, dtype, tag=name)
                src = ap if rearr is None else ap.rearrange(rearr, p=128)
                nc.sync.dma_start(t[:], src)
                return t

            wq_sb = load_const("wq", WqT, [128, 2, DM], f16, "(c p) d -> p c d")
            wk_sb = load_const("wk", WkT, [128, 2, DM], f16, "(c p) d -> p c d")
            gt_sb = load_const("gt", GTd, [128, 2, H * DM], f16, "(c p) d -> p c d")
            bq_sb = load_const("bq", bqv, [128, 2], f32, "(c p) o -> p (c o)")
            bo_sb = load_const("bo", boeff, [1, DM], f16)
            on4_sb = load_const("on4", ones4, [128, 4], f16)
            on1_sb = load_const("on1", ones1, [1, 128], f16)
            bk128_sb = load_const("bk128", blk128, [128, 128], f16)
            bk4_sb = load_const("bk4", blk4, [128, 4], f16)
            id_sb = load_const("ident", ident, [128, 128], f16)
            idx_sb = load_const("idx", idxA, [128, IDXC], i16)
            msk_sb = load_const("msk", maskA, [128, CH * 32], f32)

            qtn = ctx.enter_context(tc.tile_pool(name="qtn", bufs=1))
            qTn_sb = qtn.tile([128, 2, NBS], f16, tag="qTn")

            # ---------------- Phase A: K table + x transpose ----------------
            with ExitStack() as actx:
                qpool = actx.enter_context(tc.tile_pool(name="qtp", bufs=1))
                qt_sb = qpool.tile([128, 2, NB], f16, tag="qt")
                # split load: table group g only needs qt cols up to its rows,
                # so the first matmuls start after 1/4 of the load
                qr = qT.rearrange("(c p) b -> p c b", p=128)
                for lq in range(4):
                    sl = slice(lq * NB // 4, (lq + 1) * NB // 4)
                    nc.sync.dma_start(qt_sb[:, :, sl], qr[:, :, sl])
                qs_sb = qpool.tile([128, 2, NBS], f16, tag="qs")
                nc.sync.dma_start(qs_sb[:], qTs.rearrange("(c p) b -> p c b", p=128))

                aps = actx.enter_context(
                    tc.tile_pool(name="aps", bufs=2, space="PSUM")
                )
                astg = actx.enter_context(tc.tile_pool(name="astg", bufs=4))

                # kxtab[b, 0:256] = K rows (qt.T @ Wk.T), written into the
                # host-staged table (x half pre-filled). Strided group
                # writes: dst rows are the first 256 of each 512-elem row.
                kxwr = []
                for g in range(NBT // WG):
                    kstg = astg.tile([128, WG, DM], f16, tag="kstg")
                    for j in range(0, WG, 2):
                        kp = aps.tile([128, 2, DM], f32, tag="kp")
                        for u2 in range(2):
                            bt = g * WG + j + u2
                            for ih in range(2):
                                nc.tensor.matmul(
                                    kp[:, u2, :],
                                    lhsT=qt_sb[:, ih, bt * 128 : bt * 128 + 128],
                                    rhs=wk_sb[:, ih, :],
                                    start=(ih == 0),
                                    stop=(ih == 1),
                                )
                        nc.vector.tensor_copy(kstg[:, j : j + 2, :], kp[:])
                    rows = 128 * WG
                    kxwr.append(
                        nc.sync.dma_start(
                            kxtab[g * rows : (g + 1) * rows, 0:DM].rearrange(
                                "(j p) d -> p j d", p=128
                            ),
                            kstg[:],
                        )
                    )

                # q_T projection: qTn[d, b] = Wq @ qTs + bq  (fp16, d on
                # parts). After the table groups: it is only consumed by
                # phase B's prod muls, so it must not delay the table.
                for dh in range(2):
                    for bc in range(NBS // 512):
                        qp = aps.tile([128, 512], f32, tag="qproj")
                        for ih in range(2):
                            nc.tensor.matmul(
                                qp[:],
                                lhsT=wq_sb[:, ih, dh * 128 : dh * 128 + 128],
                                rhs=qs_sb[:, ih, bc * 512 : bc * 512 + 512],
                                start=(ih == 0),
                                stop=(ih == 1),
                            )
                        nc.scalar.activation(
                            qTn_sb[:, dh, bc * 512 : bc * 512 + 512],
                            qp[:],
                            Act.Identity,
                            bias=bq_sb[:, dh : dh + 1],
                            scale=1.0,
                        )

            # ---------------- Phase B: gather + attention chunks ----------------
            # kx bufs=4: the in-order Pool wait-queue head-blocks descgen on
            # the gather tiles' WAR edges; 4-deep keeps them stale so the 4
            # queues' descgens overlap.
            kxp = ctx.enter_context(tc.tile_pool(name="kxp", bufs=4))
            prp = ctx.enter_context(tc.tile_pool(name="prp", bufs=3))
            ktsp = ctx.enter_context(tc.tile_pool(name="ktsp", bufs=2))
            pdp = ctx.enter_context(tc.tile_pool(name="pdp", bufs=2))
            utp = ctx.enter_context(tc.tile_pool(name="utp", bufs=2))
            sfx = ctx.enter_context(tc.tile_pool(name="sfx", bufs=2))
            oup = ctx.enter_context(tc.tile_pool(name="oup", bufs=1))
            ktrp = ctx.enter_context(tc.tile_pool(name="ktrp", bufs=4, space="PSUM"))
            sczp = ctx.enter_context(tc.tile_pool(name="sczp", bufs=2, space="PSUM"))
            upsp = ctx.enter_context(tc.tile_pool(name="upsp", bufs=1, space="PSUM"))
            gop = ctx.enter_context(tc.tile_pool(name="gop", bufs=1, space="PSUM"))

            kx_t = [None] * CH
            ktr_t = [None] * CH
            pr_t = [None] * CH
            scz_t = [None] * CH

            def emit_gather(ch):
                # Two half-gathers (m 0-15 / 16-31) per chunk on different
                # queues: slice-level tile writes mean slice-level WAR
                # release, the compute chain starts on half 0 while half 1
                # is still landing, and descgens of the halves overlap.
                kx = kxp.tile([128, KN, ROW], f16, tag="kx")
                kx_t[ch] = kx
                for hf in range(2):
                    gi = nc.gpsimd.dma_gather(
                        kx[:, hf * 16 : hf * 16 + 16, :],
                        kxtab,
                        idx_sb[:, ch * 256 + hf * 128 : ch * 256 + hf * 128 + 128],
                        num_idxs=2048,
                        num_idxs_reg=2048,
                        elem_size=ROW,
                        transpose=False,
                        single_packet=False,
                        queue_num=(2 * ch + hf) % 4,
                    )
                    add_dep_helper(gi.ins, libload.ins, sync=True)
                    for w in kxwr:
                        add_dep_helper(gi.ins, w.ins, sync=True)

            def emit_tp(ch):
                # PE transpose of the K halves (f16 PSUM out) + DVE q-multiply.
                # ktr group layout: [p=d', dh, mi, (a,k)] for m = 4*g + mi.
                kx = kx_t[ch]
                ktr_t[ch] = []
                pr_t[ch] = []
                for g in range(8):
                    ktr = ktrp.tile([128, 2, 4, 128], f16, tag="ktr")
                    for dh in range(2):
                        for mi in range(4):
                            m_ = g * 4 + mi
                            nc.tensor.transpose(
                                ktr[:, dh, mi, :],
                                kx[:, m_, dh * 128 : dh * 128 + 128],
                                id_sb[:],
                            )
                    pr = prp.tile([128, 2, 4, 128], f16, tag="pr")
                    qsl = (
                        qTn_sb[:, :, ch * 128 + g * 16 : ch * 128 + g * 16 + 16]
                        .rearrange("p c (mi a u) -> p c mi a u", a=4, u=1)
                        .broadcast_to([128, 2, 4, 4, KN])
                    )
                    if g % 2:
                        # Act unloads PSUM so the DVE mul runs at the 2x f16
                        # SBUF rate (f16-PSUM reads are ~2.5x slower);
                        # alternating groups balances DVE vs Act.
                        kts = ktsp.tile([128, 2, 4, 128], f16, tag="kts")
                        nc.scalar.copy(kts[:], ktr[:])
                        ksrc = kts
                    else:
                        ksrc = ktr
                    nc.vector.tensor_mul(
                        pr[:].rearrange("p c mi (a k) -> p c mi a k", k=KN),
                        ksrc[:].rearrange("p c mi (a k) -> p c mi a k", k=KN),
                        qsl,
                    )
                    ktr_t[ch].append(ktr)
                    pr_t[ch].append(pr)

            def emit_rest(ch):
                # Per m-HALF sub-chains: softmax is over k (within a
                # partition), so m 0-15 completes scores->softmax->u without
                # waiting for m 16-31. This halves the chain latency that
                # gates the gather-tile WAR release.
                uT = utp.tile([128, 2, H, KN, 4], f16, tag="uT")
                kx = kx_t[ch]
                for half in range(2):
                    # scores: PE block-reduce over d (32-part blocks = heads)
                    scz = sczp.tile([128, 2, 16 * H], f32, tag="scz")
                    scv = scz[:, 0, :].rearrange("p (m h) -> p m h", h=H)
                    for g in range(4):
                        pr = pr_t[ch][half * 4 + g]
                        for dh in range(2):
                            for mi in range(4):
                                nc.tensor.matmul(
                                    scv[:, g * 4 + mi, dh * 4 : dh * 4 + 4],
                                    lhsT=pr[:, dh, mi, :],
                                    rhs=on4_sb[:, 0:4],
                                    start=True,
                                    stop=True,
                                )

                    # masked softmax (un-normalized exp, PE sum-bcast, recip)
                    sm = sfx.tile([128, 16, H], f32, tag="sf32")
                    mv = (
                        msk_sb[:, ch * 32 + half * 16 : ch * 32 + half * 16 + 16]
                        .rearrange("p (s u) -> p s u", u=1)
                        .broadcast_to([128, 16, H])
                    )
                    nc.vector.tensor_add(sm[:], scv, mv)
                    ex = sfx.tile([128, 16, H], f16, tag="ex")
                    nc.scalar.activation(ex[:], sm[:], Act.Exp, scale=float(SCALE))

                    nc.tensor.matmul(
                        scz[:, 1, :],
                        lhsT=bk128_sb[:],
                        rhs=ex[:].rearrange("p m h -> p (m h)"),
                        start=True,
                        stop=True,
                    )
                    rz16 = sfx.tile([128, 16 * H], f16, tag="rz16")
                    with nc.allow_low_precision(reason="softmax 1/z fits f16"):
                        nc.vector.reciprocal(rz16[:], scz[:, 1, :])
                    pn = sfx.tile([128, 16, H], f16, tag="pn")
                    nc.vector.tensor_mul(
                        pn[:], ex[:], rz16[:].rearrange("p (m h) -> p m h", h=H)
                    )

                    # Pdiag[p=(a,k), mm, a', h] = pn[p, mm, h] * (a == a')
                    pd = pdp.tile([128, 16, 4, H], f16, tag="pd")
                    nc.vector.tensor_mul(
                        pd[:],
                        pn[:].rearrange("p m (u h) -> p m u h", u=1).broadcast_to(
                            [128, 16, 4, H]
                        ),
                        bk4_sb[:]
                        .rearrange("p (u a v) -> p u a v", u=1, v=1)
                        .broadcast_to([128, 16, 4, H]),
                    )

                    # u[d', (a',h)] per (m, dh): contract (a,k) partitions
                    for dh in range(2):
                        ups = upsp.tile([128, 16, 32], f32, tag="ups")
                        for mm in range(16):
                            m_ = half * 16 + mm
                            nc.tensor.matmul(
                                ups[:, mm, :],
                                lhsT=kx[:, m_, DM + dh * 128 : DM + dh * 128 + 128],
                                rhs=pd[:, mm, :, :],
                                start=True,
                                stop=True,
                            )
                        nc.scalar.copy(
                            uT[:, dh, :, half * 16 : half * 16 + 16, :],
                            ups[:].rearrange("p s (a h) -> p h s a", h=H),
                        )

                # out = sum_h G_h u_h + boeff  (16 accumulating matmuls)
                go = gop.tile([128, DM], f32, tag="go")
                first = True
                for dh in range(2):
                    for h in range(H):
                        nc.tensor.matmul(
                            go[:],
                            lhsT=uT[:, dh, h, :, :],
                            rhs=gt_sb[:, dh, h * DM : h * DM + DM],
                            start=first,
                            stop=False,
                            skip_group_check=True,
                        )
                        first = False
                nc.tensor.matmul(
                    go[:],
                    lhsT=on1_sb[:],
                    rhs=bo_sb[:],
                    start=False,
                    stop=True,
                    skip_group_check=True,
                )
                ou = oup.tile([128, DM], f32, tag="ou")
                nc.scalar.copy(ou[:], go[:])
                nc.sync.dma_start(outp[ch * 128 : ch * 128 + 128, :], ou[:])

            # Skewed software pipeline. Per-engine stream order is what
            # matters: each engine must finish chunk c's CONSUMER work
            # (softmax/u/G on DVE+PE) before chunk c+1's PRODUCER work
            # (transposes/prod), else the streams serialize into a zigzag.
            for t in range(CH + 2):
                if t >= 2:
                    emit_rest(t - 2)
                if 1 <= t <= CH:
                    emit_tp(t - 1)
                if t < CH:
                    emit_gather(t)

    nc.compile()
    return nc


def _host_prep(query_, spatial_neighbors, mask, Wq, bq, Wk, bk, Wv, bv, Wo, bo,
               NB, NBS, ncores):
    """Pure-layout host prep: transposes, fp16 casts, index/mask relayout."""
    CH = NBS // 128
    f16 = np.float16

    q32 = np.asarray(query_, np.float32)
    qT16 = np.ascontiguousarray(q32.T).astype(f16)
    # host-staged gather table: x half filled here (pure cast/layout), K
    # half zero -- computed and written on device before any gather fires
    kxin = np.zeros((NB, 2 * DM), f16)
    kxin[:, DM:] = q32.astype(f16)
    WqT16 = np.ascontiguousarray(np.asarray(Wq, np.float32).T).astype(f16)
    WkT16 = np.ascontiguousarray(np.asarray(Wk, np.float32).T).astype(f16)
    bq32 = np.asarray(bq, np.float32).reshape(DM, 1)
    boe = (np.asarray(bo, np.float64)
           + np.asarray(Wo, np.float64) @ np.asarray(bv, np.float64))
    boe16 = boe.astype(np.float32).astype(f16).reshape(1, DM)

    # GT[d_in, h*256 + o] = (Wo_h @ Wv_h)^T = G_h^T  (exact f64 product)
    Wo64 = np.asarray(Wo, np.float64)
    Wv64 = np.asarray(Wv, np.float64)
    GT = np.empty((DM, H * DM), np.float64)
    for h in range(H):
        Gh = Wo64[:, h * DKD : (h + 1) * DKD] @ Wv64[h * DKD : (h + 1) * DKD, :]
        GT[:, h * DM : (h + 1) * DM] = Gh.T
    GT16 = GT.astype(np.float32).astype(f16)

    blkcol = np.arange(128)[:, None] // 32 == np.arange(4)[None, :]
    ones4 = blkcol.astype(f16)                      # (p//32 == j)
    ones1 = np.ones((1, 128), f16)
    blk128 = (np.arange(128)[:, None] // 32
              == np.arange(128)[None, :] // 32).astype(f16)
    blk4 = blkcol.astype(f16)
    ident = np.eye(128, dtype=f16)

    nbr = np.asarray(spatial_neighbors, np.int64)
    msk = np.asarray(mask, np.int32).reshape(NB, KN)

    def wrap16(flat):
        # flat index i at [i%16, i//16], replicated 8x for the 8 Q7 cores
        return np.tile(flat.reshape(-1, 16).T, (8, 1)).astype(np.int16)

    # gather permutation: i_local = m*128 + a*32 + k -> agent m*4+a, nbr k
    i_loc = np.arange(NBS * KN)
    chv = i_loc // 4096
    r = i_loc % 4096
    m_, a_, k_ = r // 128, (r % 128) // 32, r % 32
    bV = chv * 128 + m_ * 4 + a_

    # additive mask layout [ (a,k) partitions, (ch, s) ]: agent ch*128+s*4+a
    pa, pk = np.arange(128) // 32, np.arange(128) % 32
    chs = np.arange(CH * 32) // 32
    ss = np.arange(CH * 32) % 32

    per_core = []
    for c in range(ncores):
        base = c * NBS
        sl = slice(base, base + NBS)
        qTs16 = np.ascontiguousarray(q32[sl].T).astype(f16)

        nbr_c = nbr[sl]
        iA = wrap16(nbr_c[bV, k_])      # permuted for (a,k)-partition layout

        bM = chs[None, :] * 128 + ss[None, :] * 4 + pa[:, None]  # [128, CH*32]
        mA = np.where(msk[sl][bM, pk[:, None]] != 0, 0.0, MASK_NEG).astype(np.float32)

        per_core.append(
            dict(
                qT=qT16, qTs=qTs16, WqT=WqT16, WkT=WkT16, GTd=GT16,
                bqv=bq32, boeff=boe16, ones4=ones4, ones1=ones1,
                blk128=blk128, blk4=blk4, ident=ident,
                idxA=iA, maskA=mA, kxtab=kxin,
            )
        )
    return per_core


def kernel(**inputs):
    NB, NBS = NB_FULL, NB_FULL // NCORES
    key = (NB, NBS)
    if key not in _PROGRAM_CACHE:
        _PROGRAM_CACHE[key] = _build_program(NB, NBS)
    nc = _PROGRAM_CACHE[key]

    in_maps = _host_prep(NB=NB, NBS=NBS, ncores=NCORES, **inputs)

    from concourse.bass_utils import run_bass_kernel_spmd

    res = run_bass_kernel_spmd(nc, in_maps, list(range(NCORES)))
    out = np.concatenate([res.results[c]["out"] for c in range(NCORES)], axis=0)
    return out.reshape(NB, 1, DM).astype(np.float32)
